# revision 20
# baseline (speedup 1.0000x reference)
"""FM layer (first + second order) on 8 TRN2 NeuronCores — fp16 dma_gather v2.

Batch-parallel (512 rows/core). Table rows are fp16 padded to 32 values
(64B): [w, V^T (16), ||V||^2, pad]; a 256B block holds 4 rows. One
dma_gather per field fetches each batch row's block via int16
block-in-field indices; gathers round-robin SWDGE queues 0-3 so the
~4.5us/gather Q7 descriptor generation overlaps 4-wide across core pairs
(0,1)/(2,3)/(4,5)/(6,7) — desc-gen latency, not DMA drain, paces the
pipeline. Row-within-block (class c = idx%4) is resolved by one DVE
tensor_tensor per quarter (all-fp16, j-replicated masks keep every AP
packed for 2x mode), a pair-sum tree, and an f32 tensor_reduce over fields.
The ||V||^2 table column makes Ssq a by-product of the same reduce,
eliminating v1's Act square+accum phase. Dense part and the final combine
reuse the packed-matmul trick (u folded into the normsq column).
"""

import os
import sys

sys.path.insert(0, "/opt/trn_rl_repo")

import numpy as np

import concourse.bass as bass
import concourse.bacc as bacc
import concourse.mybir as mybir
import concourse.tile as tile
from concourse import library_config
from concourse.ap import AP

N_DENSE = 13
N_FIELDS = 26
PER_FIELD = 100000
NROWS = N_FIELDS * PER_FIELD  # device table: sparse rows only
K = 16
BATCH = 4096
N_CORES = 8
BL = BATCH // N_CORES  # 512
P = 128
T = BL // P  # 4
ROW = 18  # w, V (16), ||V||^2
PAD = 32  # fp16 values per row (64B)
RPB = 4  # rows per 256B block
NBLK = PER_FIELD // RPB  # 25000
EW = RPB * PAD  # 128 fp16 words per block (256B)
C = RPB
FB = T * EW  # 512 fp16 words per field in HQ

# quarters (compute granularity; completes in gather issue order)
FQS = [8, 8, 8, 2]
FQ0 = [0, 8, 16, 24]
# mask word offsets per quarter: sum of C*T*FQ*ROW of previous quarters
QM0 = [0]
for _r in range(3):
    QM0.append(QM0[-1] + C * T * FQS[_r] * ROW)
MSKW = QM0[-1] + C * T * FQS[3] * ROW  # 7488

KM = 2 * N_DENSE + 1  # 27
NO = ROW  # matmul rhs columns (u folded into col 17)

F32 = mybir.dt.float32
F16 = mybir.dt.float16
I16 = mybir.dt.int16


def split_multiwaits(nc: bass.Bass, max_waits: int = 1) -> int:
    """This container's walrus encodes at most one sync-wait per instruction
    (setupSyncWait raises 'Too many sync wait commands' otherwise). Hoist
    extra waits into standalone EventSemaphore ops on the same engine.
    Each hoisted op incs a dedicated dummy sem nothing waits on (CoreSim
    requires EventSemaphore instructions to carry an update)."""
    import bass_rust

    used = set()
    for func in nc.m.functions:
        for bb in func.blocks:
            for ins in bb.instructions:
                si = getattr(ins, "sync_info", None)
                if si:
                    for x in list(si.on_wait or []) + list(si.on_update or []):
                        used.add(x.id)
    dummy = None
    for num in range(max(used, default=0) + 1, 256):
        try:
            dummy = nc.alloc_semaphore("splitw_dummy", num=num)
            break
        except AssertionError:
            continue
    assert dummy is not None, "no free semaphore for splitw_dummy"
    n = 0
    for func in nc.m.functions:
        for bb in func.blocks:
            out = []
            for ins in bb.instructions:
                si = getattr(ins, "sync_info", None)
                if (
                    si is not None
                    and si.on_wait is not None
                    and len(si.on_wait) > max_waits
                ):
                    for w in list(si.on_wait[:-max_waits]):
                        n += 1
                        ev = mybir.InstEventSemaphore(
                            name=f"splitw_{n}", engine=ins.engine
                        )
                        ev.sync_info = mybir.SyncInfo(on_wait=[w], on_update=[])
                        bass_rust.then_inc(ev, dummy, 1, True)
                        out.append(ev)
                    ins.sync_info = mybir.SyncInfo(
                        on_wait=list(si.on_wait[-max_waits:]),
                        on_update=list(si.on_update or []),
                    )
                out.append(ins)
            bb.instructions = out
    return n


def build_nc() -> bass.Bass:
    nc = bacc.Bacc("TRN2", num_swdge_queues=4, dynamic_dma_scratch_size=32768)

    table = nc.dram_tensor("table", [NROWS, PAD], F16, kind="ExternalInput")
    idx = nc.dram_tensor("idx", [128, N_FIELDS * (BL // 16)], I16, kind="ExternalInput")
    dmat = nc.dram_tensor("dmat", [KM, BL + NO], F32, kind="ExternalInput")
    msk = nc.dram_tensor("msk", [128, MSKW], F16, kind="ExternalInput")
    out = nc.dram_tensor("out", [P, T], F32, kind="ExternalOutput")

    # load the gather ucode first so the IRAM swap overlaps kernel startup
    nc.gpsimd.load_library(library_config.mlp)

    with tile.TileContext(nc) as tc:
        with (
            tc.tile_pool(name="const", bufs=1) as cp,
            tc.tile_pool(name="sbuf", bufs=1) as sp,
            tc.tile_pool(name="psum", bufs=1, space="PSUM") as pp,
        ):
            # input loads ride the Activation HWDGE ring (nc.scalar) so the
            # library-reload IRAM DMA (SP ring) does not queue behind them
            idx_t = cp.tile([128, N_FIELDS * (BL // 16)], I16)
            nc.scalar.dma_start(idx_t[:], idx[:])
            dmat_t = cp.tile([KM, BL + NO], F32)
            nc.scalar.dma_start(dmat_t[:], dmat[:])
            msk_t = cp.tile([128, MSKW], F16)
            nc.scalar.dma_start(msk_t[:], msk[:])
            out_t = cp.tile([P, T], F32)

            mm_all = pp.tile([P, T * NO], F32)
            for t in range(T):
                nc.tensor.matmul(
                    mm_all[:, t * NO : (t + 1) * NO],
                    dmat_t[:, t * P : (t + 1) * P],
                    dmat_t[:, BL : BL + NO],
                    start=True,
                    stop=True,
                )

            HQ = [
                sp.tile([128, FQS[r] * FB], F16, tag=f"HQ{r}", name=f"HQ{r}")
                for r in range(4)
            ]
            # round-robin all 4 SWDGE queues: desc-gen (the ~4.5us/gather
            # serial cost per Q7 core pair) overlaps 4-wide across queue
            # pairs (0,1)/(2,3)/(4,5)/(6,7)
            for f in range(N_FIELDS):
                r = f // 8
                fr = f - FQ0[r]
                q = f % 4
                nc.gpsimd.dma_gather(
                    out_ap=HQ[r][:, fr * FB : (fr + 1) * FB].rearrange(
                        "p (s e) -> p s e", e=EW
                    ),
                    in_ap=AP(table, f * PER_FIELD * PAD, [[EW, NBLK], [1, EW]]),
                    idxs_ap=idx_t[:, f * (BL // 16) : (f + 1) * (BL // 16)],
                    num_idxs=BL,
                    num_idxs_reg=BL,
                    elem_size=EW,
                    elem_step=EW,
                    queue_num=q,
                )

            # per quarter: masked class-select (1 TT), pair-sum tree (2 TT),
            # f32 reduce over fields -> Sq [P, T*ROW]
            Sq = []
            for r in range(4):
                FQ = FQS[r]
                TF = T * FQ
                tmp = sp.tile([P, C * TF * ROW], F16, tag=f"tmp{r}", name=f"tmp{r}")
                # in0: HQ words (c, k=(f,t), j): k stride EW, c stride PAD
                in0 = AP(
                    HQ[r].tensor,
                    HQ[r][:].offset,
                    [HQ[r][:].ap[0], [PAD, C], [EW, TF], [1, ROW]],
                )
                in1 = AP(
                    msk_t.tensor,
                    msk_t[:].offset + QM0[r],
                    [msk_t[:].ap[0], [TF * ROW, C], [ROW, TF], [1, ROW]],
                )
                nc.vector.tensor_tensor(
                    out=tmp[:].rearrange("p (c x) -> p c x", c=C),
                    in0=in0,
                    in1=in1,
                    op=mybir.AluOpType.mult,
                )
                A = sp.tile([P, 2 * TF * ROW], F16, tag=f"A{r}", name=f"A{r}")
                nc.vector.tensor_tensor(
                    out=A[:],
                    in0=tmp[:, 0 : 2 * TF * ROW],
                    in1=tmp[:, 2 * TF * ROW : 4 * TF * ROW],
                    op=mybir.AluOpType.add,
                )
                B = sp.tile([P, TF * ROW], F16, tag=f"B{r}", name=f"B{r}")
                nc.vector.tensor_tensor(
                    out=B[:],
                    in0=A[:, 0 : TF * ROW],
                    in1=A[:, TF * ROW : 2 * TF * ROW],
                    op=mybir.AluOpType.add,
                )
                S = sp.tile([P, T * ROW], F32, tag=f"Sq{r}", name=f"Sq{r}")
                # B word = (f*T + t)*ROW + j ; reduce over f keeping (t, j)
                nc.vector.tensor_reduce(
                    out=S[:].rearrange("p (t j) -> p t j", t=T),
                    in_=AP(
                        B.tensor,
                        B[:].offset,
                        [B[:].ap[0], [ROW, T], [1, ROW], [T * ROW, FQ]],
                    ),
                    axis=mybir.AxisListType.X,
                    op=mybir.AluOpType.add,
                )
                Sq.append(S)

            # chain the partial adds so only the last one depends on the
            # late-finishing quarter 3
            Se01 = sp.tile([P, T * ROW], F32, tag="Se01", bufs=1)
            nc.vector.tensor_tensor(
                out=Se01[:], in0=Sq[0][:], in1=Sq[1][:], op=mybir.AluOpType.add
            )
            Se02 = sp.tile([P, T * ROW], F32, tag="Se02", bufs=1)
            nc.vector.tensor_tensor(
                out=Se02[:], in0=Se01[:], in1=Sq[2][:], op=mybir.AluOpType.add
            )
            Se = sp.tile([P, T * ROW], F32, tag="Se", bufs=1)
            nc.vector.tensor_tensor(
                out=Se[:], in0=Se02[:], in1=Sq[3][:], op=mybir.AluOpType.add
            )

            # ts = Se + mm ; per t: se2 = sum_k ts[1:17]^2 (Act square+accum)
            # out = ts[0] + 0.5*(se2 - ts[17])
            ts = sp.tile([P, T * ROW], F32, tag="ts", bufs=1)
            nc.vector.tensor_tensor(
                out=ts[:], in0=Se[:], in1=mm_all[:], op=mybir.AluOpType.add
            )
            # V columns are pre-scaled by sqrt(0.5) on the host, so
            # sum_k ts[t,1:17]^2 IS 0.5*sum_k e_k^2 — plain TT square +
            # X-reduce, no Act engine, no custom-ISA ops
            sqt = sp.tile([P, T * K], F32, tag="sqt", bufs=1)
            eap = AP(
                ts.tensor, ts[:].offset + 1, [ts[:].ap[0], [ROW, T], [1, K]]
            )
            nc.vector.tensor_tensor(
                out=sqt[:].rearrange("p (t k) -> p t k", t=T),
                in0=eap,
                in1=eap,
                op=mybir.AluOpType.mult,
            )
            se2 = sp.tile([P, T], F32, tag="se2", bufs=1)
            nc.vector.tensor_reduce(
                out=se2[:].rearrange("p (t o) -> p t o", o=1),
                in_=sqt[:].rearrange("p (t k) -> p t k", t=T),
                axis=mybir.AxisListType.X,
                op=mybir.AluOpType.add,
            )
            # table normsq column and rhs u are pre-halved, and se2 carries
            # scale=0.5 — so out = ts[0] + (se2 - ts[17]) directly
            d1 = sp.tile([P, T], F32, tag="d1", bufs=1)
            nc.vector.tensor_tensor(
                out=d1[:],
                in0=se2[:],
                in1=AP(ts.tensor, ts[:].offset + (ROW - 1), [ts[:].ap[0], [ROW, T]]),
                op=mybir.AluOpType.subtract,
            )
            nc.vector.tensor_tensor(
                out=out_t[:],
                in0=d1[:],
                in1=AP(ts.tensor, ts[:].offset, [ts[:].ap[0], [ROW, T]]),
                op=mybir.AluOpType.add,
            )
            nc.sync.dma_start(out[:], out_t[:])

    split_multiwaits(nc)
    nc.compile()
    return nc


def prepare_table(w, V):
    """fp16 device table [NROWS, 32]: row j = [w, V, ||V||^2, pad].

    Rebuilt on every call: caching on first-call contents would silently
    return stale rows if the kernel is invoked again with different w/V."""
    tb = np.zeros((NROWS, PAD), dtype=np.float16)
    Vs = V[:, N_DENSE:]
    tb[:, 0] = w[N_DENSE:, 0]
    # V columns carry sqrt(0.5) so sum_k e_k^2 comes out pre-halved
    tb[:, 1 : 1 + K] = np.sqrt(0.5) * Vs.T
    tb[:, 1 + K] = 0.5 * (Vs.astype(np.float32) ** 2).sum(axis=0)
    return tb


def prepare_inputs(dense_inputs, sparse_inputs, w0, w, V):
    dense = np.asarray(dense_inputs, dtype=np.float32)
    sparse = np.asarray(sparse_inputs, dtype=np.int64)  # [B, 26] in [0, 1e5)
    w0 = np.asarray(w0, dtype=np.float32).reshape(-1)
    w = np.asarray(w, dtype=np.float32).reshape(-1, 1)
    V = np.asarray(V, dtype=np.float32)

    table = prepare_table(w, V)

    wd = w[:N_DENSE, 0]
    Vd = V[:, :N_DENSE].T.astype(np.float32)
    u = (Vd * Vd).sum(axis=1)
    rhs = np.zeros((KM, NO), dtype=np.float32)
    rhs[:N_DENSE, 0] = wd
    rhs[:N_DENSE, 1 : 1 + K] = np.sqrt(0.5) * Vd
    rhs[N_DENSE : 2 * N_DENSE, ROW - 1] = 0.5 * u
    rhs[2 * N_DENSE, 0] = w0[0]

    blk = (sparse // RPB).astype(np.int16)  # block-in-field
    cls = (sparse % RPB).astype(np.int64)  # class (row within block)

    in_maps = []
    for core in range(N_CORES):
        dslice = dense[core * BL : (core + 1) * BL]
        dmat = np.empty((KM, BL + NO), dtype=np.float32)
        dmat[:N_DENSE, :BL] = dslice.T
        dmat[N_DENSE : 2 * N_DENSE, :BL] = (dslice * dslice).T
        dmat[2 * N_DENSE, :BL] = 1.0
        dmat[:, BL:] = rhs

        bslice = blk[core * BL : (core + 1) * BL]  # [512, 26]
        cslice = cls[core * BL : (core + 1) * BL]

        # per field: wrapped idx [128, 32]: item b -> [b%16, b//16], x8 groups
        idx_arr = np.empty((128, N_FIELDS * (BL // 16)), dtype=np.int16)
        for f in range(N_FIELDS):
            a = bslice[:, f].reshape(BL // 16, 16).T
            idx_arr[:, f * (BL // 16) : (f + 1) * (BL // 16)] = np.tile(a, (8, 1))

        # masks fp16, j-replicated: per (quarter, class) a [FQ*T, ROW] block
        # word = QM0[r] + c*TF*ROW + (f_local*T + t)*ROW + j
        cs = cslice.reshape(T, 128, N_FIELDS)  # [t, p, f]
        msk_arr = np.zeros((128, MSKW), dtype=np.float16)
        for r in range(4):
            f0, FQ = FQ0[r], FQS[r]
            TF = T * FQ
            for c in range(C):
                sel = cs[:, :, f0 : f0 + FQ] == c  # [t, p, fl]
                blk_ = sel.transpose(1, 2, 0).reshape(128, TF)  # [p, fl*T + t]
                o = QM0[r] + c * TF * ROW
                msk_arr[:, o : o + TF * ROW] = np.repeat(
                    blk_.astype(np.float16), ROW, axis=1
                )

        in_maps.append({"table": table, "idx": idx_arr, "dmat": dmat, "msk": msk_arr})
    return in_maps


def assemble_output(results):
    out = np.empty((BATCH, 1), dtype=np.float32)
    for core in range(N_CORES):
        o = results[core]["out"]
        out[core * BL : (core + 1) * BL, 0] = o.T.reshape(BL)
    return out


_NC_CACHE = None


def kernel(**inputs) -> np.ndarray:
    global _NC_CACHE
    from concourse.bass_utils import run_bass_kernel_spmd

    if _NC_CACHE is None:
        _NC_CACHE = build_nc()
    nc = _NC_CACHE
    in_maps = prepare_inputs(**inputs)
    last_err = None
    for _ in range(3):
        try:
            res = run_bass_kernel_spmd(nc, in_maps, list(range(N_CORES)))
            return assemble_output(res.results)
        except Exception as e:  # noqa: BLE001
            last_err = e
    raise last_err


# revision 23
# speedup vs baseline: 1.1162x; 1.1162x over previous
"""FM layer (first + second order) on 8 TRN2 NeuronCores — fp16 dma_gather v2.

Batch-parallel (512 rows/core). Table rows are fp16 padded to 32 values
(64B): [w, V^T (16), ||V||^2, pad]; a 256B block holds 4 rows. One
dma_gather per field fetches each batch row's block via int16
block-in-field indices; gathers round-robin SWDGE queues 0-3 so the
~4.5us/gather Q7 descriptor generation overlaps 4-wide across core pairs
(0,1)/(2,3)/(4,5)/(6,7) — desc-gen latency, not DMA drain, paces the
pipeline. Row-within-block (class c = idx%4) is resolved by one DVE
tensor_tensor per quarter (all-fp16, j-replicated masks keep every AP
packed for 2x mode), a pair-sum tree, and an f32 tensor_reduce over fields.
The ||V||^2 table column makes Ssq a by-product of the same reduce,
eliminating v1's Act square+accum phase. Dense part and the final combine
reuse the packed-matmul trick (u folded into the normsq column).
"""

import os
import sys

sys.path.insert(0, "/opt/trn_rl_repo")

import numpy as np

import concourse.bass as bass
import concourse.bacc as bacc
import concourse.mybir as mybir
import concourse.tile as tile
from concourse import library_config
from concourse.ap import AP

N_DENSE = 13
N_FIELDS = 26
PER_FIELD = 100000
NROWS = N_FIELDS * PER_FIELD  # device table: sparse rows only
K = 16
BATCH = 4096
N_CORES = 8
BL = BATCH // N_CORES  # 512
P = 128
T = BL // P  # 4
ROW = 18  # w, V (16), ||V||^2
PAD = 32  # fp16 values per row (64B)
RPB = 4  # rows per 256B block
NBLK = PER_FIELD // RPB  # 25000
EW = RPB * PAD  # 128 fp16 words per block (256B)
C = RPB
FB = T * EW  # 512 fp16 words per field in HQ

# quarters (compute granularity; completes in gather issue order)
FQS = [8, 8, 8, 2]
FQ0 = [0, 8, 16, 24]
# mask word offsets per quarter: sum of C*T*FQ*ROW of previous quarters
QM0 = [0]
for _r in range(3):
    QM0.append(QM0[-1] + C * T * FQS[_r] * ROW)
MSKW = QM0[-1] + C * T * FQS[3] * ROW  # 7488

KM = 2 * N_DENSE + 1  # 27
NO = ROW  # matmul rhs columns (u folded into col 17)

F32 = mybir.dt.float32
F16 = mybir.dt.float16
I16 = mybir.dt.int16


def split_multiwaits(nc: bass.Bass, max_waits: int = 1) -> int:
    """This container's walrus encodes at most one sync-wait per instruction
    (setupSyncWait raises 'Too many sync wait commands' otherwise). Hoist
    extra waits into standalone EventSemaphore ops on the same engine.
    Each hoisted op incs a dedicated dummy sem nothing waits on (CoreSim
    requires EventSemaphore instructions to carry an update)."""
    import bass_rust

    used = set()
    for func in nc.m.functions:
        for bb in func.blocks:
            for ins in bb.instructions:
                si = getattr(ins, "sync_info", None)
                if si:
                    for x in list(si.on_wait or []) + list(si.on_update or []):
                        used.add(x.id)
    dummy = None
    for num in range(max(used, default=0) + 1, 256):
        try:
            dummy = nc.alloc_semaphore("splitw_dummy", num=num)
            break
        except AssertionError:
            continue
    assert dummy is not None, "no free semaphore for splitw_dummy"
    n = 0
    for func in nc.m.functions:
        for bb in func.blocks:
            out = []
            for ins in bb.instructions:
                si = getattr(ins, "sync_info", None)
                if (
                    si is not None
                    and si.on_wait is not None
                    and len(si.on_wait) > max_waits
                ):
                    for w in list(si.on_wait[:-max_waits]):
                        n += 1
                        ev = mybir.InstEventSemaphore(
                            name=f"splitw_{n}", engine=ins.engine
                        )
                        ev.sync_info = mybir.SyncInfo(on_wait=[w], on_update=[])
                        bass_rust.then_inc(ev, dummy, 1, True)
                        out.append(ev)
                    ins.sync_info = mybir.SyncInfo(
                        on_wait=list(si.on_wait[-max_waits:]),
                        on_update=list(si.on_update or []),
                    )
                out.append(ins)
            bb.instructions = out
    return n


def build_nc() -> bass.Bass:
    nc = bacc.Bacc("TRN2", num_swdge_queues=4, dynamic_dma_scratch_size=32768)

    table = nc.dram_tensor("table", [NROWS, PAD], F16, kind="ExternalInput")
    idx = nc.dram_tensor("idx", [128, N_FIELDS * (BL // 16)], I16, kind="ExternalInput")
    dmat = nc.dram_tensor("dmat", [KM, BL + NO], F32, kind="ExternalInput")
    msk = nc.dram_tensor("msk", [128, MSKW], F16, kind="ExternalInput")
    out = nc.dram_tensor("out", [P, T], F32, kind="ExternalOutput")

    # load the gather ucode first so the IRAM swap overlaps kernel startup
    nc.gpsimd.load_library(library_config.mlp)

    with tile.TileContext(nc) as tc:
        with (
            tc.tile_pool(name="const", bufs=1) as cp,
            tc.tile_pool(name="sbuf", bufs=1) as sp,
            tc.tile_pool(name="psum", bufs=1, space="PSUM") as pp,
        ):
            # input loads ride the Activation HWDGE ring (nc.scalar) so the
            # library-reload IRAM DMA (SP ring) does not queue behind them
            idx_t = cp.tile([128, N_FIELDS * (BL // 16)], I16)
            nc.scalar.dma_start(idx_t[:], idx[:])
            dmat_t = cp.tile([KM, BL + NO], F32)
            nc.scalar.dma_start(dmat_t[:], dmat[:])
            msk_t = cp.tile([128, MSKW], F16)
            nc.scalar.dma_start(msk_t[:], msk[:])
            out_t = cp.tile([P, T], F32)

            mm_all = pp.tile([P, T * NO], F32)
            for t in range(T):
                nc.tensor.matmul(
                    mm_all[:, t * NO : (t + 1) * NO],
                    dmat_t[:, t * P : (t + 1) * P],
                    dmat_t[:, BL : BL + NO],
                    start=True,
                    stop=True,
                )

            HQ = [
                sp.tile([128, FQS[r] * FB], F16, tag=f"HQ{r}", name=f"HQ{r}")
                for r in range(4)
            ]
            # round-robin all 4 SWDGE queues: desc-gen (the ~4.5us/gather
            # serial cost per Q7 core pair) overlaps 4-wide across queue
            # pairs (0,1)/(2,3)/(4,5)/(6,7)
            for f in range(N_FIELDS):
                r = f // 8
                fr = f - FQ0[r]
                q = f % 4
                nc.gpsimd.dma_gather(
                    out_ap=HQ[r][:, fr * FB : (fr + 1) * FB].rearrange(
                        "p (s e) -> p s e", e=EW
                    ),
                    in_ap=AP(table, f * PER_FIELD * PAD, [[EW, NBLK], [1, EW]]),
                    idxs_ap=idx_t[:, f * (BL // 16) : (f + 1) * (BL // 16)],
                    num_idxs=BL,
                    num_idxs_reg=BL,
                    elem_size=EW,
                    elem_step=EW,
                    queue_num=q,
                )

            # per quarter: masked class-select (1 TT), pair-sum tree (2 TT),
            # f32 reduce over fields -> Sq [P, T*ROW]
            Sq = []
            for r in range(4):
                FQ = FQS[r]
                TF = T * FQ
                tmp = sp.tile([P, C * TF * ROW], F16, tag=f"tmp{r}", name=f"tmp{r}")
                # in0: HQ words (c, k=(f,t), j): k stride EW, c stride PAD
                in0 = AP(
                    HQ[r].tensor,
                    HQ[r][:].offset,
                    [HQ[r][:].ap[0], [PAD, C], [EW, TF], [1, ROW]],
                )
                in1 = AP(
                    msk_t.tensor,
                    msk_t[:].offset + QM0[r],
                    [msk_t[:].ap[0], [TF * ROW, C], [ROW, TF], [1, ROW]],
                )
                nc.vector.tensor_tensor(
                    out=tmp[:].rearrange("p (c x) -> p c x", c=C),
                    in0=in0,
                    in1=in1,
                    op=mybir.AluOpType.mult,
                )
                A = sp.tile([P, 2 * TF * ROW], F16, tag=f"A{r}", name=f"A{r}")
                nc.vector.tensor_tensor(
                    out=A[:],
                    in0=tmp[:, 0 : 2 * TF * ROW],
                    in1=tmp[:, 2 * TF * ROW : 4 * TF * ROW],
                    op=mybir.AluOpType.add,
                )
                # B written in [t, j, f] layout so the field-reduce reads a
                # packed inner dim (fp16 2x mode) instead of a strided one
                B = sp.tile([P, TF * ROW], F16, tag=f"B{r}", name=f"B{r}")
                nc.vector.tensor_tensor(
                    out=AP(
                        B.tensor,
                        B[:].offset,
                        [B[:].ap[0], [1, FQ], [ROW * FQ, T], [FQ, ROW]],
                    ),
                    in0=AP(
                        A.tensor,
                        A[:].offset,
                        [A[:].ap[0], [T * ROW, FQ], [ROW, T], [1, ROW]],
                    ),
                    in1=AP(
                        A.tensor,
                        A[:].offset + TF * ROW,
                        [A[:].ap[0], [T * ROW, FQ], [ROW, T], [1, ROW]],
                    ),
                    op=mybir.AluOpType.add,
                )
                S = sp.tile([P, T * ROW], F16, tag=f"Sq{r}", name=f"Sq{r}")
                with nc.allow_low_precision(
                    reason="<=8-field fp16 partial sums; final Se add is f32"
                ):
                    nc.vector.tensor_reduce(
                        out=S[:].rearrange("p (t j) -> p t j", t=T),
                        in_=AP(
                            B.tensor,
                            B[:].offset,
                            [B[:].ap[0], [ROW * FQ, T], [FQ, ROW], [1, FQ]],
                        ),
                        axis=mybir.AxisListType.X,
                        op=mybir.AluOpType.add,
                    )
                Sq.append(S)

            # chain the partial adds so only the last one depends on the
            # late-finishing quarter 3
            Se01 = sp.tile([P, T * ROW], F16, tag="Se01", bufs=1)
            nc.vector.tensor_tensor(
                out=Se01[:], in0=Sq[0][:], in1=Sq[1][:], op=mybir.AluOpType.add
            )
            Se02 = sp.tile([P, T * ROW], F16, tag="Se02", bufs=1)
            nc.vector.tensor_tensor(
                out=Se02[:], in0=Se01[:], in1=Sq[2][:], op=mybir.AluOpType.add
            )
            Se = sp.tile([P, T * ROW], F32, tag="Se", bufs=1)
            nc.vector.tensor_tensor(
                out=Se[:], in0=Se02[:], in1=Sq[3][:], op=mybir.AluOpType.add
            )

            # ts = Se + mm ; per t: se2 = sum_k ts[1:17]^2 (Act square+accum)
            # out = ts[0] + 0.5*(se2 - ts[17])
            ts = sp.tile([P, T * ROW], F32, tag="ts", bufs=1)
            nc.vector.tensor_tensor(
                out=ts[:], in0=Se[:], in1=mm_all[:], op=mybir.AluOpType.add
            )
            # V columns are pre-scaled by sqrt(0.5) on the host, so
            # sum_k ts[t,1:17]^2 IS 0.5*sum_k e_k^2 — plain TT square +
            # X-reduce, no Act engine, no custom-ISA ops
            sqt = sp.tile([P, T * K], F32, tag="sqt", bufs=1)
            eap = AP(
                ts.tensor, ts[:].offset + 1, [ts[:].ap[0], [ROW, T], [1, K]]
            )
            nc.vector.tensor_tensor(
                out=sqt[:].rearrange("p (t k) -> p t k", t=T),
                in0=eap,
                in1=eap,
                op=mybir.AluOpType.mult,
            )
            se2 = sp.tile([P, T], F32, tag="se2", bufs=1)
            nc.vector.tensor_reduce(
                out=se2[:].rearrange("p (t o) -> p t o", o=1),
                in_=sqt[:].rearrange("p (t k) -> p t k", t=T),
                axis=mybir.AxisListType.X,
                op=mybir.AluOpType.add,
            )
            # table normsq column and rhs u are pre-halved, and se2 carries
            # scale=0.5 — so out = ts[0] + (se2 - ts[17]) directly
            d1 = sp.tile([P, T], F32, tag="d1", bufs=1)
            nc.vector.tensor_tensor(
                out=d1[:],
                in0=se2[:],
                in1=AP(ts.tensor, ts[:].offset + (ROW - 1), [ts[:].ap[0], [ROW, T]]),
                op=mybir.AluOpType.subtract,
            )
            nc.vector.tensor_tensor(
                out=out_t[:],
                in0=d1[:],
                in1=AP(ts.tensor, ts[:].offset, [ts[:].ap[0], [ROW, T]]),
                op=mybir.AluOpType.add,
            )
            nc.sync.dma_start(out[:], out_t[:])

    split_multiwaits(nc)
    nc.compile()
    return nc


def prepare_table(w, V):
    """fp16 device table [NROWS, 32]: row j = [w, V, ||V||^2, pad].

    Rebuilt on every call: caching on first-call contents would silently
    return stale rows if the kernel is invoked again with different w/V."""
    tb = np.zeros((NROWS, PAD), dtype=np.float16)
    Vs = V[:, N_DENSE:]
    tb[:, 0] = w[N_DENSE:, 0]
    # V columns carry sqrt(0.5) so sum_k e_k^2 comes out pre-halved
    tb[:, 1 : 1 + K] = np.sqrt(0.5) * Vs.T
    tb[:, 1 + K] = 0.5 * (Vs.astype(np.float32) ** 2).sum(axis=0)
    return tb


def prepare_inputs(dense_inputs, sparse_inputs, w0, w, V):
    dense = np.asarray(dense_inputs, dtype=np.float32)
    sparse = np.asarray(sparse_inputs, dtype=np.int64)  # [B, 26] in [0, 1e5)
    w0 = np.asarray(w0, dtype=np.float32).reshape(-1)
    w = np.asarray(w, dtype=np.float32).reshape(-1, 1)
    V = np.asarray(V, dtype=np.float32)

    table = prepare_table(w, V)

    wd = w[:N_DENSE, 0]
    Vd = V[:, :N_DENSE].T.astype(np.float32)
    u = (Vd * Vd).sum(axis=1)
    rhs = np.zeros((KM, NO), dtype=np.float32)
    rhs[:N_DENSE, 0] = wd
    rhs[:N_DENSE, 1 : 1 + K] = np.sqrt(0.5) * Vd
    rhs[N_DENSE : 2 * N_DENSE, ROW - 1] = 0.5 * u
    rhs[2 * N_DENSE, 0] = w0[0]

    blk = (sparse // RPB).astype(np.int16)  # block-in-field
    cls = (sparse % RPB).astype(np.int64)  # class (row within block)

    in_maps = []
    for core in range(N_CORES):
        dslice = dense[core * BL : (core + 1) * BL]
        dmat = np.empty((KM, BL + NO), dtype=np.float32)
        dmat[:N_DENSE, :BL] = dslice.T
        dmat[N_DENSE : 2 * N_DENSE, :BL] = (dslice * dslice).T
        dmat[2 * N_DENSE, :BL] = 1.0
        dmat[:, BL:] = rhs

        bslice = blk[core * BL : (core + 1) * BL]  # [512, 26]
        cslice = cls[core * BL : (core + 1) * BL]

        # per field: wrapped idx [128, 32]: item b -> [b%16, b//16], x8 groups
        idx_arr = np.empty((128, N_FIELDS * (BL // 16)), dtype=np.int16)
        for f in range(N_FIELDS):
            a = bslice[:, f].reshape(BL // 16, 16).T
            idx_arr[:, f * (BL // 16) : (f + 1) * (BL // 16)] = np.tile(a, (8, 1))

        # masks fp16, j-replicated: per (quarter, class) a [FQ*T, ROW] block
        # word = QM0[r] + c*TF*ROW + (f_local*T + t)*ROW + j
        cs = cslice.reshape(T, 128, N_FIELDS)  # [t, p, f]
        msk_arr = np.zeros((128, MSKW), dtype=np.float16)
        for r in range(4):
            f0, FQ = FQ0[r], FQS[r]
            TF = T * FQ
            for c in range(C):
                sel = cs[:, :, f0 : f0 + FQ] == c  # [t, p, fl]
                blk_ = sel.transpose(1, 2, 0).reshape(128, TF)  # [p, fl*T + t]
                o = QM0[r] + c * TF * ROW
                msk_arr[:, o : o + TF * ROW] = np.repeat(
                    blk_.astype(np.float16), ROW, axis=1
                )

        in_maps.append({"table": table, "idx": idx_arr, "dmat": dmat, "msk": msk_arr})
    return in_maps


def assemble_output(results):
    out = np.empty((BATCH, 1), dtype=np.float32)
    for core in range(N_CORES):
        o = results[core]["out"]
        out[core * BL : (core + 1) * BL, 0] = o.T.reshape(BL)
    return out


_NC_CACHE = None


def kernel(**inputs) -> np.ndarray:
    global _NC_CACHE
    from concourse.bass_utils import run_bass_kernel_spmd

    if _NC_CACHE is None:
        _NC_CACHE = build_nc()
    nc = _NC_CACHE
    in_maps = prepare_inputs(**inputs)
    last_err = None
    for _ in range(3):
        try:
            res = run_bass_kernel_spmd(nc, in_maps, list(range(N_CORES)))
            return assemble_output(res.results)
        except Exception as e:  # noqa: BLE001
            last_err = e
    raise last_err


# revision 24
# speedup vs baseline: 1.1645x; 1.0432x over previous
"""FM layer (first + second order) on 8 TRN2 NeuronCores — fp16 dma_gather v2.

Batch-parallel (512 rows/core). Table rows are fp16 padded to 32 values
(64B): [w, V^T (16), ||V||^2, pad]; a 256B block holds 4 rows. One
dma_gather per field fetches each batch row's block via int16
block-in-field indices; gathers round-robin SWDGE queues 0-3 so the
~4.5us/gather Q7 descriptor generation overlaps 4-wide across core pairs
(0,1)/(2,3)/(4,5)/(6,7) — desc-gen latency, not DMA drain, paces the
pipeline. Row-within-block (class c = idx%4) is resolved by one DVE
tensor_tensor per quarter (all-fp16, j-replicated masks keep every AP
packed for 2x mode), a pair-sum tree, and an f32 tensor_reduce over fields.
The ||V||^2 table column makes Ssq a by-product of the same reduce,
eliminating v1's Act square+accum phase. Dense part and the final combine
reuse the packed-matmul trick (u folded into the normsq column).
"""

import os
import sys

sys.path.insert(0, "/opt/trn_rl_repo")

import numpy as np

import concourse.bass as bass
import concourse.bacc as bacc
import concourse.mybir as mybir
import concourse.tile as tile
from concourse import library_config
from concourse.ap import AP

N_DENSE = 13
N_FIELDS = 26
PER_FIELD = 100000
NROWS = N_FIELDS * PER_FIELD  # device table: sparse rows only
K = 16
BATCH = 4096
N_CORES = 8
BL = BATCH // N_CORES  # 512
P = 128
T = BL // P  # 4
ROW = 18  # w, V (16), ||V||^2
PAD = 32  # fp16 values per row (64B)
RPB = 4  # rows per 256B block
NBLK = PER_FIELD // RPB  # 25000
EW = RPB * PAD  # 128 fp16 words per block (256B)
C = RPB
FB = T * EW  # 512 fp16 words per field in HQ

# quarters (compute granularity; completes in gather issue order)
FQS = [8, 8, 8, 2]
FQ0 = [0, 8, 16, 24]
# mask word offsets per quarter: sum of C*T*FQ*ROW of previous quarters
QM0 = [0]
for _r in range(3):
    QM0.append(QM0[-1] + C * T * FQS[_r] * ROW)
MSKW = QM0[-1] + C * T * FQS[3] * ROW  # 7488

KM = 2 * N_DENSE + 1  # 27
NO = ROW  # matmul rhs columns (u folded into col 17)

F32 = mybir.dt.float32
F16 = mybir.dt.float16
I16 = mybir.dt.int16


def split_multiwaits(nc: bass.Bass, max_waits: int = 1) -> int:
    """This container's walrus encodes at most one sync-wait per instruction
    (setupSyncWait raises 'Too many sync wait commands' otherwise). Hoist
    extra waits into standalone EventSemaphore ops on the same engine.
    Each hoisted op incs a dedicated dummy sem nothing waits on (CoreSim
    requires EventSemaphore instructions to carry an update)."""
    import bass_rust

    used = set()
    for func in nc.m.functions:
        for bb in func.blocks:
            for ins in bb.instructions:
                si = getattr(ins, "sync_info", None)
                if si:
                    for x in list(si.on_wait or []) + list(si.on_update or []):
                        used.add(x.id)
    dummy = None
    for num in range(max(used, default=0) + 1, 256):
        try:
            dummy = nc.alloc_semaphore("splitw_dummy", num=num)
            break
        except AssertionError:
            continue
    assert dummy is not None, "no free semaphore for splitw_dummy"
    n = 0
    for func in nc.m.functions:
        for bb in func.blocks:
            out = []
            for ins in bb.instructions:
                si = getattr(ins, "sync_info", None)
                if (
                    si is not None
                    and si.on_wait is not None
                    and len(si.on_wait) > max_waits
                ):
                    for w in list(si.on_wait[:-max_waits]):
                        n += 1
                        ev = mybir.InstEventSemaphore(
                            name=f"splitw_{n}", engine=ins.engine
                        )
                        ev.sync_info = mybir.SyncInfo(on_wait=[w], on_update=[])
                        bass_rust.then_inc(ev, dummy, 1, True)
                        out.append(ev)
                    ins.sync_info = mybir.SyncInfo(
                        on_wait=list(si.on_wait[-max_waits:]),
                        on_update=list(si.on_update or []),
                    )
                out.append(ins)
            bb.instructions = out
    return n


def build_nc() -> bass.Bass:
    nc = bacc.Bacc("TRN2", num_swdge_queues=4, dynamic_dma_scratch_size=32768)

    table = nc.dram_tensor("table", [NROWS, PAD], F16, kind="ExternalInput")
    idx = nc.dram_tensor("idx", [128, N_FIELDS * (BL // 16)], I16, kind="ExternalInput")
    dmat = nc.dram_tensor("dmat", [KM, BL + NO], F32, kind="ExternalInput")
    msk = nc.dram_tensor("msk", [128, MSKW], F16, kind="ExternalInput")
    out = nc.dram_tensor("out", [P, T], F32, kind="ExternalOutput")

    # load the gather ucode first so the IRAM swap overlaps kernel startup
    nc.gpsimd.load_library(library_config.mlp)

    with tile.TileContext(nc) as tc:
        with (
            tc.tile_pool(name="const", bufs=1) as cp,
            tc.tile_pool(name="sbuf", bufs=1) as sp,
            tc.tile_pool(name="psum", bufs=1, space="PSUM") as pp,
        ):
            # input loads ride the Activation HWDGE ring (nc.scalar) so the
            # library-reload IRAM DMA (SP ring) does not queue behind them
            idx_t = cp.tile([128, N_FIELDS * (BL // 16)], I16)
            nc.scalar.dma_start(idx_t[:], idx[:])
            dmat_t = cp.tile([KM, BL + NO], F32)
            nc.scalar.dma_start(dmat_t[:], dmat[:])
            msk_t = cp.tile([128, MSKW], F16)
            nc.scalar.dma_start(msk_t[:], msk[:])
            out_t = cp.tile([P, T], F32)

            mm_all = pp.tile([P, T * NO], F32)
            for t in range(T):
                nc.tensor.matmul(
                    mm_all[:, t * NO : (t + 1) * NO],
                    dmat_t[:, t * P : (t + 1) * P],
                    dmat_t[:, BL : BL + NO],
                    start=True,
                    stop=True,
                )

            HQ = [
                sp.tile([128, FQS[r] * FB], F16, tag=f"HQ{r}", name=f"HQ{r}")
                for r in range(4)
            ]
            # round-robin all 4 SWDGE queues: desc-gen (the ~4.5us/gather
            # serial cost per Q7 core pair) overlaps 4-wide across queue
            # pairs (0,1)/(2,3)/(4,5)/(6,7)
            for f in range(N_FIELDS):
                r = f // 8
                fr = f - FQ0[r]
                q = f % 4
                nc.gpsimd.dma_gather(
                    out_ap=HQ[r][:, fr * FB : (fr + 1) * FB].rearrange(
                        "p (s e) -> p s e", e=EW
                    ),
                    in_ap=AP(table, f * PER_FIELD * PAD, [[EW, NBLK], [1, EW]]),
                    idxs_ap=idx_t[:, f * (BL // 16) : (f + 1) * (BL // 16)],
                    num_idxs=BL,
                    num_idxs_reg=BL,
                    elem_size=EW,
                    elem_step=EW,
                    queue_num=q,
                )

            # per quarter: masked class-select (1 TT), pair-sum tree (2 TT),
            # f32 reduce over fields -> Sq [P, T*ROW]
            Sq = []
            for r in range(4):
                FQ = FQS[r]
                TF = T * FQ
                tmp = sp.tile([P, C * TF * ROW], F16, tag=f"tmp{r}", name=f"tmp{r}")
                # in0: HQ words (c, k=(f,t), j): k stride EW, c stride PAD
                in0 = AP(
                    HQ[r].tensor,
                    HQ[r][:].offset,
                    [HQ[r][:].ap[0], [PAD, C], [EW, TF], [1, ROW]],
                )
                in1 = AP(
                    msk_t.tensor,
                    msk_t[:].offset + QM0[r],
                    [msk_t[:].ap[0], [TF * ROW, C], [ROW, TF], [1, ROW]],
                )
                nc.vector.tensor_tensor(
                    out=tmp[:].rearrange("p (c x) -> p c x", c=C),
                    in0=in0,
                    in1=in1,
                    op=mybir.AluOpType.mult,
                )
                A = sp.tile([P, 2 * TF * ROW], F16, tag=f"A{r}", name=f"A{r}")
                nc.vector.tensor_tensor(
                    out=A[:],
                    in0=tmp[:, 0 : 2 * TF * ROW],
                    in1=tmp[:, 2 * TF * ROW : 4 * TF * ROW],
                    op=mybir.AluOpType.add,
                )
                B = sp.tile([P, TF * ROW], F16, tag=f"B{r}", name=f"B{r}")
                nc.vector.tensor_tensor(
                    out=B[:],
                    in0=A[:, 0 : TF * ROW],
                    in1=A[:, TF * ROW : 2 * TF * ROW],
                    op=mybir.AluOpType.add,
                )
                S = sp.tile([P, T * ROW], F32, tag=f"Sq{r}", name=f"Sq{r}")
                # B word = (f*T + t)*ROW + j ; reduce over f keeping (t, j)
                nc.vector.tensor_reduce(
                    out=S[:].rearrange("p (t j) -> p t j", t=T),
                    in_=AP(
                        B.tensor,
                        B[:].offset,
                        [B[:].ap[0], [ROW, T], [1, ROW], [T * ROW, FQ]],
                    ),
                    axis=mybir.AxisListType.X,
                    op=mybir.AluOpType.add,
                )
                Sq.append(S)

            # chain the partial adds so only the last one depends on the
            # late-finishing quarter 3
            Se01 = sp.tile([P, T * ROW], F32, tag="Se01", bufs=1)
            nc.vector.tensor_tensor(
                out=Se01[:], in0=Sq[0][:], in1=Sq[1][:], op=mybir.AluOpType.add
            )
            Se02 = sp.tile([P, T * ROW], F32, tag="Se02", bufs=1)
            nc.vector.tensor_tensor(
                out=Se02[:], in0=Se01[:], in1=Sq[2][:], op=mybir.AluOpType.add
            )
            Se = sp.tile([P, T * ROW], F32, tag="Se", bufs=1)
            nc.vector.tensor_tensor(
                out=Se[:], in0=Se02[:], in1=Sq[3][:], op=mybir.AluOpType.add
            )

            # ts = Se + mm ; per t: se2 = sum_k ts[1:17]^2 (Act square+accum)
            # out = ts[0] + 0.5*(se2 - ts[17])
            ts = sp.tile([P, T * ROW], F32, tag="ts", bufs=1)
            nc.vector.tensor_tensor(
                out=ts[:], in0=Se[:], in1=mm_all[:], op=mybir.AluOpType.add
            )
            # V columns are pre-scaled by sqrt(0.5) on the host, so
            # sum_k ts[t,1:17]^2 IS 0.5*sum_k e_k^2 — plain TT square +
            # X-reduce, no Act engine, no custom-ISA ops
            sqt = sp.tile([P, T * K], F32, tag="sqt", bufs=1)
            eap = AP(
                ts.tensor, ts[:].offset + 1, [ts[:].ap[0], [ROW, T], [1, K]]
            )
            nc.vector.tensor_tensor(
                out=sqt[:].rearrange("p (t k) -> p t k", t=T),
                in0=eap,
                in1=eap,
                op=mybir.AluOpType.mult,
            )
            se2 = sp.tile([P, T], F32, tag="se2", bufs=1)
            nc.vector.tensor_reduce(
                out=se2[:].rearrange("p (t o) -> p t o", o=1),
                in_=sqt[:].rearrange("p (t k) -> p t k", t=T),
                axis=mybir.AxisListType.X,
                op=mybir.AluOpType.add,
            )
            # table normsq column and rhs u are pre-halved, and se2 carries
            # scale=0.5 — so out = ts[0] + (se2 - ts[17]) directly
            d1 = sp.tile([P, T], F32, tag="d1", bufs=1)
            nc.vector.tensor_tensor(
                out=d1[:],
                in0=se2[:],
                in1=AP(ts.tensor, ts[:].offset + (ROW - 1), [ts[:].ap[0], [ROW, T]]),
                op=mybir.AluOpType.subtract,
            )
            nc.vector.tensor_tensor(
                out=out_t[:],
                in0=d1[:],
                in1=AP(ts.tensor, ts[:].offset, [ts[:].ap[0], [ROW, T]]),
                op=mybir.AluOpType.add,
            )
            nc.sync.dma_start(out[:], out_t[:])

    split_multiwaits(nc)
    nc.compile()
    return nc


def prepare_table(w, V):
    """fp16 device table [NROWS, 32]: row j = [w, V, ||V||^2, pad].

    Rebuilt on every call: caching on first-call contents would silently
    return stale rows if the kernel is invoked again with different w/V."""
    tb = np.zeros((NROWS, PAD), dtype=np.float16)
    Vs = V[:, N_DENSE:]
    tb[:, 0] = w[N_DENSE:, 0]
    # V columns carry sqrt(0.5) so sum_k e_k^2 comes out pre-halved
    tb[:, 1 : 1 + K] = np.sqrt(0.5) * Vs.T
    tb[:, 1 + K] = 0.5 * (Vs.astype(np.float32) ** 2).sum(axis=0)
    return tb


def prepare_inputs(dense_inputs, sparse_inputs, w0, w, V):
    dense = np.asarray(dense_inputs, dtype=np.float32)
    sparse = np.asarray(sparse_inputs, dtype=np.int64)  # [B, 26] in [0, 1e5)
    w0 = np.asarray(w0, dtype=np.float32).reshape(-1)
    w = np.asarray(w, dtype=np.float32).reshape(-1, 1)
    V = np.asarray(V, dtype=np.float32)

    table = prepare_table(w, V)

    wd = w[:N_DENSE, 0]
    Vd = V[:, :N_DENSE].T.astype(np.float32)
    u = (Vd * Vd).sum(axis=1)
    rhs = np.zeros((KM, NO), dtype=np.float32)
    rhs[:N_DENSE, 0] = wd
    rhs[:N_DENSE, 1 : 1 + K] = np.sqrt(0.5) * Vd
    rhs[N_DENSE : 2 * N_DENSE, ROW - 1] = 0.5 * u
    rhs[2 * N_DENSE, 0] = w0[0]

    blk = (sparse // RPB).astype(np.int16)  # block-in-field
    cls = (sparse % RPB).astype(np.int64)  # class (row within block)

    in_maps = []
    for core in range(N_CORES):
        dslice = dense[core * BL : (core + 1) * BL]
        dmat = np.empty((KM, BL + NO), dtype=np.float32)
        dmat[:N_DENSE, :BL] = dslice.T
        dmat[N_DENSE : 2 * N_DENSE, :BL] = (dslice * dslice).T
        dmat[2 * N_DENSE, :BL] = 1.0
        dmat[:, BL:] = rhs

        bslice = blk[core * BL : (core + 1) * BL]  # [512, 26]
        cslice = cls[core * BL : (core + 1) * BL]

        # per field: wrapped idx [128, 32]: item b -> [b%16, b//16], x8 groups
        idx_arr = np.empty((128, N_FIELDS * (BL // 16)), dtype=np.int16)
        for f in range(N_FIELDS):
            a = bslice[:, f].reshape(BL // 16, 16).T
            idx_arr[:, f * (BL // 16) : (f + 1) * (BL // 16)] = np.tile(a, (8, 1))

        # masks fp16, j-replicated: per (quarter, class) a [FQ*T, ROW] block
        # word = QM0[r] + c*TF*ROW + (f_local*T + t)*ROW + j
        cs = cslice.reshape(T, 128, N_FIELDS)  # [t, p, f]
        msk_arr = np.zeros((128, MSKW), dtype=np.float16)
        for r in range(4):
            f0, FQ = FQ0[r], FQS[r]
            TF = T * FQ
            for c in range(C):
                sel = cs[:, :, f0 : f0 + FQ] == c  # [t, p, fl]
                blk_ = sel.transpose(1, 2, 0).reshape(128, TF)  # [p, fl*T + t]
                o = QM0[r] + c * TF * ROW
                msk_arr[:, o : o + TF * ROW] = np.repeat(
                    blk_.astype(np.float16), ROW, axis=1
                )

        in_maps.append({"table": table, "idx": idx_arr, "dmat": dmat, "msk": msk_arr})
    return in_maps


def assemble_output(results):
    out = np.empty((BATCH, 1), dtype=np.float32)
    for core in range(N_CORES):
        o = results[core]["out"]
        out[core * BL : (core + 1) * BL, 0] = o.T.reshape(BL)
    return out


_NC_CACHE = None


def kernel(**inputs) -> np.ndarray:
    global _NC_CACHE
    from concourse.bass_utils import run_bass_kernel_spmd

    if _NC_CACHE is None:
        _NC_CACHE = build_nc()
    nc = _NC_CACHE
    in_maps = prepare_inputs(**inputs)
    last_err = None
    for _ in range(3):
        try:
            res = run_bass_kernel_spmd(nc, in_maps, list(range(N_CORES)))
            return assemble_output(res.results)
        except Exception as e:  # noqa: BLE001
            last_err = e
    raise last_err
